# revision 3
# baseline (speedup 1.0000x reference)
"""Trainium2 Bass kernel for the low-rank slot Elman RNN.

Problem:
    per step t:  Wx = x_t @ W_x.T                      [B, D]
                 Vh_s = h_s @ V_s.T ; Uh_s = Vh_s @ U_s.T   (per slot, low rank)
                 h_s  = tanh(Wx + Uh_s + b)            [B, D] per slot
                 out  = (sum_s C_s h_s) * silu(z_t)    [B, D]
    outputs: out [T, B, D], h [T+1, S, B, D]

Strategy (8 NeuronCores):
  * slot sharding: core s owns slot s's recurrence for the full batch
    (the 8 slot recurrences are independent given Wx; per-core per-step
    tensor-engine weight traffic is U_s+V_s only).
  * Wx+b precomputed on every core with one big fp32 matmul (weights
    stationary, x^T moving), staged to DRAM in the step-loop tile layout.
  * step loop: all matmuls weights-stationary in bf16 (fast weight load),
    h kept as [d-on-partitions, (dchunk, b)] tiles so tanh/adds are cheap.
  * out-combine: each core scales its h by C_s, streams it to DRAM, and
    chunked ReduceScatter collectives (TOPSP/SDMA, overlapped with the
    step loop) produce sum_s C_s h_s; each core then applies silu(z) to
    its t-stripes.
  * all layout transposes (x^T, W_x^T, V^T, U^T) are host-side prep.
"""

import numpy as np
import ml_dtypes

# problem dims (hardcoded per contest contract)
D, S, R, T, B = 1024, 8, 256, 1024, 8
P = 128                 # partitions
DC, RC = D // P, R // P  # 8 d-chunks, 2 r-chunks
CB = DC * B             # 64 columns in an h tile: col = dchunk*B + b
NCORES = 8
CHUNK = 128             # timesteps per ReduceScatter chunk
NCH = T // CHUNK        # number of collectives
STRIDE = CHUNK // NCORES  # 16 timesteps per rank per chunk
TSH = NCH * STRIDE      # 128 output timesteps per core
WT = 64                 # Wx phase: timesteps per tile (N = WT*B = 512)

BF16 = ml_dtypes.bfloat16


def _t_indices(core):
    """Timestep indices owned by `core` for the `out` output (RS striping)."""
    k = np.arange(NCH)[:, None]
    j = np.arange(STRIDE)[None, :]
    return (k * CHUNK + core * STRIDE + j).reshape(-1)


def _build():
    import concourse.mybir as mybir
    import concourse.tile as tile
    from concourse import bacc

    f32 = mybir.dt.float32
    bf16 = mybir.dt.bfloat16
    AF = mybir.ActivationFunctionType

    nc = bacc.Bacc("TRN2", target_bir_lowering=False, debug=False,
                   num_devices=NCORES)

    # ---- kernel I/O ----
    xT = nc.dram_tensor("xT", [D, T * B], f32, kind="ExternalInput")
    wxT = nc.dram_tensor("wxT", [D, D], f32, kind="ExternalInput")
    vT = nc.dram_tensor("vT", [D, R], bf16, kind="ExternalInput")
    uT = nc.dram_tensor("uT", [R, D], bf16, kind="ExternalInput")
    bt = nc.dram_tensor("bt", [P, DC], f32, kind="ExternalInput")
    cst = nc.dram_tensor("cst", [P, 1], f32, kind="ExternalInput")
    h0f = nc.dram_tensor("h0f", [P, CB], f32, kind="ExternalInput")
    h0b = nc.dram_tensor("h0b", [P, CB], bf16, kind="ExternalInput")
    zt = nc.dram_tensor("zt", [TSH, P, CB], f32, kind="ExternalInput")
    hsh = nc.dram_tensor("hsh", [T + 1, P, CB], f32, kind="ExternalOutput")
    osh = nc.dram_tensor("osh", [TSH, P, CB], f32, kind="ExternalOutput")

    # ---- internal DRAM ----
    wxb = nc.dram_tensor("wxb", [T, P, CB], f32, kind="Internal")
    csum_in = nc.dram_tensor("csum_in", [T, P, CB], f32, kind="Internal")
    csum_out = nc.dram_tensor("csum_out", [NCH, STRIDE, P, CB], f32,
                              kind="Internal")

    rg = [list(range(NCORES))]

    with tile.TileContext(nc) as tc:
        with tc.tile_pool(name="const", bufs=1) as constp:
            vT_sb = constp.tile([P, DC, R], bf16)
            nc.sync.dma_start(vT_sb[:], vT.rearrange("(dc p) r -> p dc r", p=P))
            uT_sb = constp.tile([P, RC, D], bf16)
            nc.sync.dma_start(uT_sb[:], uT.rearrange("(rc p) d -> p rc d", p=P))
            b_sb = constp.tile([P, DC], f32)
            nc.sync.dma_start(b_sb[:], bt[:])
            c_sb = constp.tile([P, 1], f32)
            nc.sync.dma_start(c_sb[:], cst[:])

            # ================= Wx precompute phase =================
            with (
                tc.tile_pool(name="wxw", bufs=1) as wxwp,
                tc.tile_pool(name="wxx", bufs=3) as xp,
                tc.tile_pool(name="wxs", bufs=3) as stp,
                tc.tile_pool(name="wxps", bufs=2, space="PSUM") as wxps,
            ):
                wxT_sb = wxwp.tile([P, DC, D], f32)
                nc.sync.dma_start(wxT_sb[:],
                                  wxT.rearrange("(kc p) d -> p kc d", p=P))
                xTr = xT.rearrange("(kc p) n -> p kc n", p=P)
                for tt in range(T // WT):
                    x_sb = xp.tile([P, DC, WT * B], f32)
                    nc.sync.dma_start(
                        x_sb[:], xTr[:, :, tt * WT * B:(tt + 1) * WT * B])
                    stage = stp.tile([P, WT, CB], f32)
                    for mc in range(DC):
                        pw = wxps.tile([P, WT * B], f32)
                        for kc in range(DC):
                            nc.tensor.matmul(
                                pw[:],
                                wxT_sb[:, kc, mc * P:(mc + 1) * P],
                                x_sb[:, kc, :],
                                start=(kc == 0), stop=(kc == DC - 1))
                        # stage[:, t, mc*B+b] = pw[:, t*B+b] + bias[d]
                        nc.scalar.activation(
                            stage[:, :, mc * B:(mc + 1) * B],
                            pw.rearrange("p (t b) -> p t b", b=B),
                            AF.Identity,
                            bias=b_sb[:, mc:mc + 1])
                    nc.sync.dma_start(
                        wxb[tt * WT:(tt + 1) * WT].rearrange("t p c -> p t c"),
                        stage[:])

            # ================= recurrence =================
            with (
                tc.tile_pool(name="wl", bufs=3) as wlp,
                tc.tile_pool(name="hb", bufs=2) as hbp,
                tc.tile_pool(name="vh", bufs=2) as vhp,
                tc.tile_pool(name="h32", bufs=3) as h32p,
                tc.tile_pool(name="pc", bufs=3) as pcp,
                tc.tile_pool(name="pv", bufs=2, space="PSUM") as pvp,
                tc.tile_pool(name="pu", bufs=2, space="PSUM") as pup,
            ):
                nc.sync.dma_start(hsh[0], h0f[:])
                hbf_prev = hbp.tile([P, CB], bf16)
                nc.sync.dma_start(hbf_prev[:], h0b[:])

                wxb_sb = None
                for t in range(T):
                    if t % 8 == 0:
                        wxb_sb = wlp.tile([P, 8, CB], f32)
                        nc.sync.dma_start(
                            wxb_sb[:],
                            wxb[t:t + 8].rearrange("t p c -> p t c"))
                    # phase A: VhT[r, b] += V[r, d] * h[d, b]
                    pv = pvp.tile([P, RC * B], f32)
                    for rc in range(RC):
                        for dc in range(DC):
                            nc.tensor.matmul(
                                pv[:, rc * B:(rc + 1) * B],
                                vT_sb[:, dc, rc * P:(rc + 1) * P],
                                hbf_prev[:, dc * B:(dc + 1) * B],
                                start=(dc == 0), stop=(dc == DC - 1))
                    vh = vhp.tile([P, RC * B], bf16)
                    nc.vector.tensor_copy(vh[:], pv[:])
                    # phase B: UhT[d, b] += U[d, r] * Vh[r, b]
                    pu = pup.tile([P, CB], f32)
                    for dc in range(DC):
                        for rc in range(RC):
                            nc.tensor.matmul(
                                pu[:, dc * B:(dc + 1) * B],
                                uT_sb[:, rc, dc * P:(dc + 1) * P],
                                vh[:, rc * B:(rc + 1) * B],
                                start=(rc == 0), stop=(rc == RC - 1))
                    # h = tanh(Uh + (Wx + b))
                    nc.vector.tensor_add(pu[:], pu[:], wxb_sb[:, t % 8, :])
                    hbf = hbp.tile([P, CB], bf16)
                    nc.scalar.activation(hbf[:], pu[:], AF.Tanh)
                    h32 = h32p.tile([P, CB], f32)
                    nc.scalar.activation(h32[:], pu[:], AF.Tanh)
                    pc = pcp.tile([P, CB], f32)
                    nc.vector.tensor_scalar_mul(pc[:], h32[:], c_sb[:])
                    nc.sync.dma_start(hsh[t + 1], h32[:])
                    nc.sync.dma_start(csum_in[t], pc[:])
                    hbf_prev = hbf
                    if (t + 1) % CHUNK == 0:
                        k = t // CHUNK
                        nc.gpsimd.collective_compute(
                            "ReduceScatter",
                            mybir.AluOpType.add,
                            replica_groups=rg,
                            ins=[csum_in[k * CHUNK:(k + 1) * CHUNK]],
                            outs=[csum_out[k]])

            # ================= out = silu(z) * hsum =================
            with (
                tc.tile_pool(name="cz", bufs=3) as czp,
                tc.tile_pool(name="ch", bufs=3) as chp,
                tc.tile_pool(name="co", bufs=3) as cop,
            ):
                G = 8  # timesteps per combine tile
                for k in range(NCH):
                    for g in range(STRIDE // G):
                        r0 = k * STRIDE + g * G
                        hz = chp.tile([P, G, CB], f32)
                        nc.sync.dma_start(
                            hz[:],
                            csum_out[k, g * G:(g + 1) * G].rearrange(
                                "t p c -> p t c"))
                        zz = czp.tile([P, G, CB], f32)
                        nc.sync.dma_start(
                            zz[:],
                            zt[r0:r0 + G].rearrange("t p c -> p t c"))
                        sz = cop.tile([P, G, CB], f32)
                        nc.scalar.activation(sz[:], zz[:], AF.Silu)
                        oo = cop.tile([P, G, CB], f32)
                        nc.vector.tensor_mul(oo[:], sz[:], hz[:])
                        nc.sync.dma_start(
                            osh[r0:r0 + G].rearrange("t p c -> p t c"), oo[:])

    nc.compile()
    return nc


_NC_CACHE = {}
_LAST_IN_MAPS = None


def _get_nc():
    if "nc" not in _NC_CACHE:
        _NC_CACHE["nc"] = _build()
    return _NC_CACHE["nc"]


def _pack_bdp(a):
    """[..., B, D] -> [..., P, CB] tile layout: tile[p, dc*B+b] = a[b, dc*P+p]."""
    lead = a.shape[:-2]
    a = a.reshape(lead + (B, DC, P))
    order = tuple(range(len(lead))) + (len(lead) + 2, len(lead) + 1, len(lead))
    return np.ascontiguousarray(a.transpose(order)).reshape(lead + (P, CB))


def _unpack_bdp(a):
    """[..., P, CB] -> [..., B, D]."""
    lead = a.shape[:-2]
    a = a.reshape(lead + (P, DC, B))
    order = tuple(range(len(lead))) + (len(lead) + 2, len(lead) + 1, len(lead))
    return np.ascontiguousarray(a.transpose(order)).reshape(lead + (B, D))


def kernel(x, z, h0, W_x, U, V, b, C):
    from concourse.bass_utils import run_bass_kernel_spmd

    x = np.asarray(x, np.float32)
    z = np.asarray(z, np.float32)
    h0 = np.asarray(h0, np.float32)
    W_x = np.asarray(W_x, np.float32)
    U = np.asarray(U, np.float32)
    V = np.asarray(V, np.float32)
    b = np.asarray(b, np.float32)
    C = np.asarray(C, np.float32)

    nc = _get_nc()

    xT_np = np.ascontiguousarray(x.reshape(T * B, D).T)
    wxT_np = np.ascontiguousarray(W_x.T)
    bt_np = np.ascontiguousarray(b.reshape(DC, P).T)

    in_maps = []
    for s in range(NCORES):
        tix = _t_indices(s)
        in_maps.append({
            "xT": xT_np,
            "wxT": wxT_np,
            "vT": np.ascontiguousarray(V[s].T).astype(BF16),
            "uT": np.ascontiguousarray(U[s].T).astype(BF16),
            "bt": bt_np,
            "cst": np.full((P, 1), C[s], np.float32),
            "h0f": _pack_bdp(h0[:, s, :]),
            "h0b": _pack_bdp(h0[:, s, :]).astype(BF16),
            "zt": _pack_bdp(z[tix]),
        })

    global _LAST_IN_MAPS
    _LAST_IN_MAPS = in_maps
    res = run_bass_kernel_spmd(nc, in_maps, core_ids=list(range(NCORES)))

    h = np.empty((T + 1, S, B, D), np.float32)
    out = np.empty((T, B, D), np.float32)
    for s in range(NCORES):
        h[:, s] = _unpack_bdp(res.results[s]["hsh"])
        out[_t_indices(s)] = _unpack_bdp(res.results[s]["osh"])
    return out, h


# revision 7
# speedup vs baseline: 1.1124x; 1.1124x over previous
"""Trainium2 Bass kernel for the low-rank slot Elman RNN.

Problem:
    per step t:  Wx = x_t @ W_x.T                       [B, D]
                 Uh_s = (U_s @ V_s) h_s   (low-rank slot update)
                 h_s  = tanh(Wx + Uh_s + b)             [B, D] per slot
                 out  = (sum_s C_s h_s) * silu(z_t)     [B, D]
    outputs: out [T, B, D], h [T+1, S, B, D]

Strategy (8 NeuronCores, slot sharding):
  * core s owns slot s's recurrence for the full batch (slot recurrences
    are independent given Wx; only the final combine couples them).
  * Ws = (U_s V_s)^T is merged on-chip once (bf16), so each timestep is a
    single matmul phase: 64 [128x128]x[128x8] weight-stationary MMs.
  * Wx+b is preloaded into PSUM (DVE/ACT write, has_written bits kept set
    by never using start=True in the loop) so the MMs accumulate straight
    onto it and tanh reads PSUM -> one cross-engine hop per step.
  * tanh is split in halves so the first half is ready while the PE still
    streams the second half's matmuls.
  * Wx precompute is sharded over cores (each computes T/8 steps, plus a
    redundant first shard so the loop can start immediately) and merged
    with one AllGather that overlaps the first 128 steps.
  * out-combine: each core scales h by C_s, streams to DRAM; chunked
    ReduceScatter collectives overlap the loop; silu(z)*sum applied per
    rank's time-stripes right after each RS; tapered last chunks keep the
    exposed tail small.
"""

import numpy as np
import ml_dtypes

# problem dims (hardcoded per contest contract)
D, S, R, T, B = 1024, 8, 256, 1024, 8
P = 128
DC, RC = D // P, R // P  # 8, 2
CB = DC * B              # 64 cols in an h tile: col = dchunk*B + b
NCORES = 8
WT = 64                  # Wx phase: timesteps per tile (N = WT*B = 512)
TSHARD = T // NCORES     # 128: Wx steps computed per core
# ReduceScatter chunking (timesteps per collective); tapered tail
CHUNKS = [128] * 6 + [64, 64, 64, 32, 32]
assert sum(CHUNKS) == T
TSH = sum(c // NCORES for c in CHUNKS)  # out rows per core

BF16 = ml_dtypes.bfloat16


def _chunk_offsets():
    offs = []
    o = 0
    for c in CHUNKS:
        offs.append(o)
        o += c
    return offs


def _t_indices(core):
    """Timestep indices owned by `core` for the `out` output (RS striping)."""
    idx = []
    for off, ln in zip(_chunk_offsets(), CHUNKS):
        st = ln // NCORES
        idx.extend(range(off + core * st, off + (core + 1) * st))
    return np.array(idx)


def _build():
    import concourse.mybir as mybir
    import concourse.tile as tile
    from concourse import bacc

    f32 = mybir.dt.float32
    bf16 = mybir.dt.bfloat16
    AF = mybir.ActivationFunctionType

    nc = bacc.Bacc("TRN2", target_bir_lowering=False, debug=False,
                   num_devices=NCORES)

    # ---- kernel I/O ----
    # x^T slices for the Wx phase: shard 0 (redundant on all cores) + own
    xT0 = nc.dram_tensor("xT0", [D, TSHARD * B], f32, kind="ExternalInput")
    xTm = nc.dram_tensor("xTm", [D, TSHARD * B], f32, kind="ExternalInput")
    wxT = nc.dram_tensor("wxT", [D, D], f32, kind="ExternalInput")
    vnat = nc.dram_tensor("vnat", [R, D], bf16, kind="ExternalInput")  # V[s]
    uT = nc.dram_tensor("uT", [R, D], bf16, kind="ExternalInput")      # U[s].T
    bt = nc.dram_tensor("bt", [P, DC], f32, kind="ExternalInput")
    cst = nc.dram_tensor("cst", [P, 1], f32, kind="ExternalInput")
    h0b = nc.dram_tensor("h0b", [P, CB], bf16, kind="ExternalInput")
    zt = nc.dram_tensor("zt", [TSH, P, CB], f32, kind="ExternalInput")
    hsh = nc.dram_tensor("hsh", [T, P, CB], bf16, kind="ExternalOutput")
    osh = nc.dram_tensor("osh", [TSH, P, CB], f32, kind="ExternalOutput")

    # ---- internal DRAM ----
    wxb0 = nc.dram_tensor("wxb0", [TSHARD, P, CB], f32, kind="Internal")
    wxbm = nc.dram_tensor("wxbm", [TSHARD, P, CB], f32, kind="Internal")
    wxb = nc.dram_tensor("wxb", [T, P, CB], f32, kind="Internal",
                         addr_space="Shared")
    csin = [nc.dram_tensor(f"csin{k}", [ln, P, CB], f32, kind="Internal")
            for k, ln in enumerate(CHUNKS)]
    csout = [nc.dram_tensor(f"csout{k}", [ln // NCORES, P, CB], f32,
                            kind="Internal")
             for k, ln in enumerate(CHUNKS)]

    rg = [list(range(NCORES))]

    with tile.TileContext(nc) as tc:
        with tc.tile_pool(name="const", bufs=1) as constp:
            b_sb = constp.tile([P, DC], f32)
            nc.sync.dma_start(b_sb[:], bt[:])
            c_sb = constp.tile([P, 1], f32)
            nc.sync.dma_start(c_sb[:], cst[:])
            zero_bf = constp.tile([P, CB], bf16)
            nc.vector.memset(zero_bf[:], 0.0)
            # merged slot weight, transposed: WsT[d_in, d_out] = Ws[d_out, d_in]
            wsT_sb = constp.tile([P, DC, D], bf16)

            # ============ WsT = V^T-contracted merge (one-time) ============
            with (
                tc.tile_pool(name="mw", bufs=1) as mwp,
                tc.tile_pool(name="mps", bufs=2, space="PSUM") as mps,
            ):
                v_sb = mwp.tile([P, RC, D], bf16)
                nc.sync.dma_start(v_sb[:], vnat.rearrange("(rc p) d -> p rc d", p=P))
                uT_sb = mwp.tile([P, RC, D], bf16)
                nc.sync.dma_start(uT_sb[:], uT.rearrange("(rc p) d -> p rc d", p=P))
                for di in range(DC):
                    for nh in range(2):
                        pm = mps.tile([P, 512], f32)
                        for rc in range(RC):
                            nc.tensor.matmul(
                                pm[:],
                                v_sb[:, rc, di * P:(di + 1) * P],
                                uT_sb[:, rc, nh * 512:(nh + 1) * 512],
                                start=(rc == 0), stop=(rc == RC - 1))
                        nc.scalar.activation(
                            wsT_sb[:, di, nh * 512:(nh + 1) * 512], pm[:],
                            AF.Copy)

            # ============ Wx phase: shard 0 + own shard, then AllGather ====
            with (
                tc.tile_pool(name="wxw", bufs=1) as wxwp,
                tc.tile_pool(name="wxx", bufs=2) as xp,
                tc.tile_pool(name="wxs", bufs=2) as stp,
                tc.tile_pool(name="wxps", bufs=2, space="PSUM") as wxps,
            ):
                wxT_sb = wxwp.tile([P, DC, D], f32)
                nc.sync.dma_start(wxT_sb[:],
                                  wxT.rearrange("(kc p) d -> p kc d", p=P))

                def wx_pass(src, dst):
                    srcr = src.rearrange("(kc p) n -> p kc n", p=P)
                    for tt in range(TSHARD // WT):
                        x_sb = xp.tile([P, DC, WT * B], f32)
                        nc.sync.dma_start(
                            x_sb[:], srcr[:, :, tt * WT * B:(tt + 1) * WT * B])
                        stage = stp.tile([P, WT, CB], f32)
                        for mc in range(DC):
                            pw = wxps.tile([P, WT * B], f32)
                            for kc in range(DC):
                                nc.tensor.matmul(
                                    pw[:],
                                    wxT_sb[:, kc, mc * P:(mc + 1) * P],
                                    x_sb[:, kc, :],
                                    start=(kc == 0), stop=(kc == DC - 1))
                            nc.scalar.activation(
                                stage[:, :, mc * B:(mc + 1) * B],
                                pw.rearrange("p (t b) -> p t b", b=B),
                                AF.Identity,
                                bias=b_sb[:, mc:mc + 1])
                        nc.scalar.dma_start(
                            dst[tt * WT:(tt + 1) * WT].rearrange(
                                "t p c -> p t c"),
                            stage[:])

                wx_pass(xT0, wxb0)
                wx_pass(xTm, wxbm)
                nc.gpsimd.collective_compute(
                    "AllGather", mybir.AluOpType.bypass, replica_groups=rg,
                    ins=[wxbm[:]], outs=[wxb[:]])

            # ===================== recurrence =====================
            with (
                tc.tile_pool(name="wl", bufs=3) as wlp,
                tc.tile_pool(name="hb", bufs=3) as hbp,
                tc.tile_pool(name="pc", bufs=3) as pcp,
                tc.tile_pool(name="pua", bufs=2, space="PSUM") as puap,
                tc.tile_pool(name="pub", bufs=2, space="PSUM") as pubp,
                tc.tile_pool(name="cz", bufs=2) as czp,
                tc.tile_pool(name="ch", bufs=2) as chp,
                tc.tile_pool(name="co", bufs=2) as cop,
            ):
                H = CB // 2  # 32 cols per half (dchunks 0-3 / 4-7)
                # psum tiles, statically rotated; bootstrap has_written bits
                pua = [puap.tile([P, H], f32, name=f"pua{i}") for i in range(2)]
                pub = [pubp.tile([P, H], f32, name=f"pub{i}") for i in range(2)]
                for pt in (*pua, *pub):
                    nc.tensor.matmul(pt[:], wsT_sb[:, 0, 0:P], zero_bf[:, 0:H],
                                     start=True, stop=True)

                hbf = hbp.tile([P, CB], bf16, name="hinit")
                nc.sync.dma_start(hbf[:], h0b[:])

                wxb_sb = None
                ck = 0          # current RS chunk
                ck_off = 0      # its start step
                for t in range(T):
                    if t % 8 == 0:
                        wxb_sb = wlp.tile([P, 8, CB], f32)
                        src = (wxb0[t:t + 8] if t + 8 <= TSHARD
                               else wxb[t:t + 8])
                        nc.sync.dma_start(wxb_sb[:],
                                          src.rearrange("t p c -> p t c"))
                    puA, puB = pua[t % 2], pub[t % 2]
                    ws = wxb_sb[:, t % 8, :]
                    # preload Wx+b into PSUM (MMs accumulate onto it)
                    nc.scalar.activation(puA[:], ws[:, 0:H], AF.Copy)
                    nc.vector.tensor_copy(puB[:], ws[:, H:CB])
                    # 64 weight-stationary MMs: dc_out-major so half A of the
                    # new h is finished while half B still streams
                    newh = hbp.tile([P, CB], bf16, name="newh")
                    for half, pu in ((0, puA), (1, puB)):
                        for do in range(half * 4, half * 4 + 4):
                            out = pu[:, (do % 4) * B:(do % 4 + 1) * B]
                            for di in range(DC):
                                nc.tensor.matmul(
                                    out,
                                    wsT_sb[:, di, do * P:(do + 1) * P],
                                    hbf[:, di * B:(di + 1) * B],
                                    start=False, stop=(di == DC - 1),
                                    skip_group_check=True)
                        nc.scalar.activation(
                            newh[:, half * H:(half + 1) * H], pu[:], AF.Tanh)
                    pc = pcp.tile([P, CB], f32)
                    nc.vector.tensor_scalar_mul(pc[:], newh[:], c_sb[:])
                    nc.scalar.dma_start(hsh[t], newh[:])
                    nc.scalar.dma_start(csin[ck][t - ck_off], pc[:])
                    hbf = newh
                    if t - ck_off + 1 == CHUNKS[ck]:
                        nc.gpsimd.collective_compute(
                            "ReduceScatter", mybir.AluOpType.add,
                            replica_groups=rg,
                            ins=[csin[ck][:]], outs=[csout[ck][:]])
                        # combine for this chunk: out = silu(z) * hsum
                        st = CHUNKS[ck] // NCORES
                        r0 = sum(c // NCORES for c in CHUNKS[:ck])
                        for g0 in range(0, st, 8):
                            g = min(8, st - g0)
                            hz = chp.tile([P, 8, CB], f32, name="hz")
                            nc.sync.dma_start(
                                hz[:, :g, :],
                                csout[ck][g0:g0 + g].rearrange("t p c -> p t c"))
                            zz = czp.tile([P, 8, CB], f32, name="zz")
                            nc.sync.dma_start(
                                zz[:, :g, :],
                                zt[r0 + g0:r0 + g0 + g].rearrange(
                                    "t p c -> p t c"))
                            sz = cop.tile([P, 8, CB], f32, name="sz")
                            nc.scalar.activation(sz[:, :g, :], zz[:, :g, :],
                                                 AF.Silu)
                            oo = cop.tile([P, 8, CB], f32, name="oo")
                            nc.vector.tensor_mul(oo[:, :g, :], sz[:, :g, :],
                                                 hz[:, :g, :])
                            nc.scalar.dma_start(
                                osh[r0 + g0:r0 + g0 + g].rearrange(
                                    "t p c -> p t c"),
                                oo[:, :g, :])
                        ck_off += CHUNKS[ck]
                        ck += 1

    nc.compile()
    return nc


_NC_CACHE = {}
_LAST_IN_MAPS = None


def _get_nc():
    if "nc" not in _NC_CACHE:
        _NC_CACHE["nc"] = _build()
    return _NC_CACHE["nc"]


def _pack_bdp(a):
    """[..., B, D] -> [..., P, CB] tile layout: tile[p, dc*B+b] = a[b, dc*P+p]."""
    lead = a.shape[:-2]
    a = a.reshape(lead + (B, DC, P))
    n = len(lead)
    order = tuple(range(n)) + (n + 2, n + 1, n)
    return np.ascontiguousarray(a.transpose(order)).reshape(lead + (P, CB))


def _unpack_bdp(a):
    """[..., P, CB] -> [..., B, D]."""
    lead = a.shape[:-2]
    a = a.reshape(lead + (P, DC, B))
    n = len(lead)
    order = tuple(range(n)) + (n + 2, n + 1, n)
    return np.ascontiguousarray(a.transpose(order)).reshape(lead + (B, D))


def kernel(x, z, h0, W_x, U, V, b, C):
    from concourse.bass_utils import run_bass_kernel_spmd

    x = np.asarray(x, np.float32)
    z = np.asarray(z, np.float32)
    h0 = np.asarray(h0, np.float32)
    W_x = np.asarray(W_x, np.float32)
    U = np.asarray(U, np.float32)
    V = np.asarray(V, np.float32)
    b = np.asarray(b, np.float32)
    C = np.asarray(C, np.float32)

    nc = _get_nc()

    xT_np = np.ascontiguousarray(x.reshape(T * B, D).T)
    wxT_np = np.ascontiguousarray(W_x.T)
    bt_np = np.ascontiguousarray(b.reshape(DC, P).T)

    in_maps = []
    for s in range(NCORES):
        tix = _t_indices(s)
        in_maps.append({
            "xT0": np.ascontiguousarray(xT_np[:, :TSHARD * B]),
            "xTm": np.ascontiguousarray(
                xT_np[:, s * TSHARD * B:(s + 1) * TSHARD * B]),
            "wxT": wxT_np,
            "vnat": V[s].astype(BF16),
            "uT": np.ascontiguousarray(U[s].T).astype(BF16),
            "bt": bt_np,
            "cst": np.full((P, 1), C[s], np.float32),
            "h0b": _pack_bdp(h0[:, s, :]).astype(BF16),
            "zt": _pack_bdp(z[tix]),
        })

    global _LAST_IN_MAPS
    _LAST_IN_MAPS = in_maps
    res = run_bass_kernel_spmd(nc, in_maps, core_ids=list(range(NCORES)))

    h = np.empty((T + 1, S, B, D), np.float32)
    out = np.empty((T, B, D), np.float32)
    for s in range(NCORES):
        h[0, s] = h0[:, s, :]
        h[1:, s] = _unpack_bdp(res.results[s]["hsh"].astype(np.float32))
        out[_t_indices(s)] = _unpack_bdp(res.results[s]["osh"])
    return out, h


# revision 8
# speedup vs baseline: 1.3476x; 1.2115x over previous
"""Trainium2 Bass kernel for the low-rank slot Elman RNN.

Problem:
    per step t:  Wx = x_t @ W_x.T                       [B, D]
                 Uh_s = (U_s @ V_s) h_s   (low-rank slot update)
                 h_s  = tanh(Wx + Uh_s + b)             [B, D] per slot
                 out  = (sum_s C_s h_s) * silu(z_t)     [B, D]
    outputs: out [T, B, D], h [T+1, S, B, D]

Strategy (8 NeuronCores, slot sharding):
  * core s owns slot s's recurrence for the full batch (slot recurrences
    are independent given Wx; only the final combine couples them).
  * Ws = (U_s V_s)^T is merged on-chip once (bf16), so each timestep is a
    single matmul phase: 64 [128x128]x[128x8] weight-stationary MMs.
  * Wx+b is preloaded into PSUM (DVE writes, has_written bits kept set by
    never using start=True in the loop) so the MMs accumulate straight
    onto it and tanh reads PSUM directly -> one cross-engine hop per step.
  * MMs ordered in two d_in blocks and tanh split in halves so each half
    of the new h is ready while the PE still streams the other block.
  * h and C_s*h stores are batched 8 steps at a time from staging tiles
    (DMA issue cost is per-descriptor on the issuing engine).
  * Wx precompute is sharded over cores (each computes T/8 steps, plus a
    redundant first shard so the loop can start immediately) and merged
    with one AllGather that overlaps the first 128 steps.
  * out-combine: chunked ReduceScatter overlaps the loop; silu(z)*sum is
    applied after the loop (cheap), so only the last chunk's RS is
    exposed.
"""

import numpy as np
import ml_dtypes

# problem dims (hardcoded per contest contract)
D, S, R, T, B = 1024, 8, 256, 1024, 8
P = 128
DC, RC = D // P, R // P  # 8, 2
CB = DC * B              # 64 cols in an h tile: col = dchunk*B + b
H = CB // 2              # half-tile columns
NCORES = 8
WT = 64                  # Wx phase: timesteps per tile (N = WT*B = 512)
TSHARD = T // NCORES     # 128: Wx steps computed per core
G = 8                    # timesteps per store-group
CHUNKS = [128] * 6 + [64] * 4   # ReduceScatter chunking (timesteps)
assert sum(CHUNKS) == T
TSH = sum(c // NCORES for c in CHUNKS)  # out rows per core

BF16 = ml_dtypes.bfloat16


def _t_indices(core):
    """Timestep indices owned by `core` for the `out` output (RS striping)."""
    idx = []
    off = 0
    for ln in CHUNKS:
        st = ln // NCORES
        idx.extend(range(off + core * st, off + (core + 1) * st))
        off += ln
    return np.array(idx)


def _build():
    import concourse.mybir as mybir
    import concourse.tile as tile
    from concourse import bacc

    f32 = mybir.dt.float32
    bf16 = mybir.dt.bfloat16
    AF = mybir.ActivationFunctionType

    nc = bacc.Bacc("TRN2", target_bir_lowering=False, debug=False,
                   num_devices=NCORES)

    # ---- kernel I/O ----
    xT0 = nc.dram_tensor("xT0", [D, TSHARD * B], f32, kind="ExternalInput")
    xTm = nc.dram_tensor("xTm", [D, TSHARD * B], f32, kind="ExternalInput")
    wxT = nc.dram_tensor("wxT", [D, D], f32, kind="ExternalInput")
    vnat = nc.dram_tensor("vnat", [R, D], bf16, kind="ExternalInput")  # V[s]
    uT = nc.dram_tensor("uT", [R, D], bf16, kind="ExternalInput")      # U[s].T
    bt = nc.dram_tensor("bt", [P, DC], f32, kind="ExternalInput")
    cst = nc.dram_tensor("cst", [P, 1], f32, kind="ExternalInput")
    h0b = nc.dram_tensor("h0b", [P, CB], bf16, kind="ExternalInput")
    zt = nc.dram_tensor("zt", [TSH // G, P, G, CB], f32, kind="ExternalInput")
    hsh = nc.dram_tensor("hsh", [P, T, CB], bf16, kind="ExternalOutput")
    osh = nc.dram_tensor("osh", [TSH // G, P, G, CB], f32,
                         kind="ExternalOutput")

    # ---- internal DRAM ----
    wxb0 = nc.dram_tensor("wxb0", [TSHARD, P, CB], f32, kind="Internal")
    wxbm = nc.dram_tensor("wxbm", [TSHARD, P, CB], f32, kind="Internal")
    wxb = nc.dram_tensor("wxb", [T, P, CB], f32, kind="Internal",
                         addr_space="Shared")
    # collective buffers, grouped [n_groups, P, G, CB] with dim0 = time-major
    csin = [nc.dram_tensor(f"csin{k}", [ln // G, P, G, CB], f32,
                           kind="Internal")
            for k, ln in enumerate(CHUNKS)]
    csout = [nc.dram_tensor(f"csout{k}", [ln // G // NCORES, P, G, CB], f32,
                            kind="Internal")
             for k, ln in enumerate(CHUNKS)]

    rg = [list(range(NCORES))]

    with tile.TileContext(nc) as tc:
        with tc.tile_pool(name="const", bufs=1) as constp:
            b_sb = constp.tile([P, DC], f32)
            nc.sync.dma_start(b_sb[:], bt[:])
            c_sb = constp.tile([P, 1], f32)
            nc.sync.dma_start(c_sb[:], cst[:])
            zero_bf = constp.tile([P, H], bf16)
            nc.vector.memset(zero_bf[:], 0.0)
            # merged slot weight, transposed: WsT[d_in, d_out] = Ws[d_out, d_in]
            wsT_sb = constp.tile([P, DC, D], bf16)

            # ============ WsT merge (one-time) ============
            with (
                tc.tile_pool(name="mw", bufs=1) as mwp,
                tc.tile_pool(name="mps", bufs=2, space="PSUM") as mps,
            ):
                v_sb = mwp.tile([P, RC, D], bf16)
                nc.sync.dma_start(v_sb[:],
                                  vnat.rearrange("(rc p) d -> p rc d", p=P))
                uT_sb = mwp.tile([P, RC, D], bf16)
                nc.sync.dma_start(uT_sb[:],
                                  uT.rearrange("(rc p) d -> p rc d", p=P))
                for di in range(DC):
                    for nh in range(2):
                        pm = mps.tile([P, 512], f32)
                        for rc in range(RC):
                            nc.tensor.matmul(
                                pm[:],
                                v_sb[:, rc, di * P:(di + 1) * P],
                                uT_sb[:, rc, nh * 512:(nh + 1) * 512],
                                start=(rc == 0), stop=(rc == RC - 1))
                        nc.scalar.activation(
                            wsT_sb[:, di, nh * 512:(nh + 1) * 512], pm[:],
                            AF.Copy)

            # ============ Wx phase: shard 0 + own shard, then AllGather ====
            with (
                tc.tile_pool(name="wxw", bufs=1) as wxwp,
                tc.tile_pool(name="wxx", bufs=2) as xp,
                tc.tile_pool(name="wxs", bufs=2) as stp,
                tc.tile_pool(name="wxps", bufs=2, space="PSUM") as wxps,
            ):
                wxT_sb = wxwp.tile([P, DC, D], f32)
                nc.sync.dma_start(wxT_sb[:],
                                  wxT.rearrange("(kc p) d -> p kc d", p=P))

                def wx_pass(src, dst):
                    srcr = src.rearrange("(kc p) n -> p kc n", p=P)
                    for tt in range(TSHARD // WT):
                        x_sb = xp.tile([P, DC, WT * B], f32)
                        nc.sync.dma_start(
                            x_sb[:], srcr[:, :, tt * WT * B:(tt + 1) * WT * B])
                        stage = stp.tile([P, WT, CB], f32)
                        for mc in range(DC):
                            pw = wxps.tile([P, WT * B], f32)
                            for kc in range(DC):
                                nc.tensor.matmul(
                                    pw[:],
                                    wxT_sb[:, kc, mc * P:(mc + 1) * P],
                                    x_sb[:, kc, :],
                                    start=(kc == 0), stop=(kc == DC - 1))
                            nc.scalar.activation(
                                stage[:, :, mc * B:(mc + 1) * B],
                                pw.rearrange("p (t b) -> p t b", b=B),
                                AF.Identity,
                                bias=b_sb[:, mc:mc + 1])
                        nc.sync.dma_start(
                            dst[tt * WT:(tt + 1) * WT].rearrange(
                                "t p c -> p t c"),
                            stage[:])

                wx_pass(xT0, wxb0)
                wx_pass(xTm, wxbm)
                nc.gpsimd.collective_compute(
                    "AllGather", mybir.AluOpType.bypass, replica_groups=rg,
                    ins=[wxbm[:]], outs=[wxb[:]])

            # ===================== recurrence =====================
            with (
                tc.tile_pool(name="wl", bufs=3) as wlp,
                tc.tile_pool(name="hs", bufs=3) as hsp,
                tc.tile_pool(name="ps", bufs=3) as psp,
                tc.tile_pool(name="pua", bufs=2, space="PSUM") as puap,
                tc.tile_pool(name="pub", bufs=2, space="PSUM") as pubp,
            ):
                # psum tiles, statically rotated; bootstrap has_written bits
                pua = [puap.tile([P, H], f32, name=f"pua{i}") for i in range(2)]
                pub = [pubp.tile([P, H], f32, name=f"pub{i}") for i in range(2)]
                for pt in (*pua, *pub):
                    nc.tensor.matmul(pt[:], wsT_sb[:, 0, 0:P], zero_bf[:],
                                     start=True, stop=True)

                hinit = hsp.tile([P, 1, CB], bf16, name="hinit")
                nc.sync.dma_start(hinit[:, 0, :], h0b[:])
                hbf = hinit[:, 0, :]

                wxb_sb = None
                hstage = None
                pstage = None
                ck = 0          # current RS chunk
                ck_off = 0      # its start step
                for t in range(T):
                    if t % G == 0:
                        wxb_sb = wlp.tile([P, G, CB], f32, name="wxl")
                        src = (wxb0[t:t + G] if t + G <= TSHARD
                               else wxb[t:t + G])
                        nc.sync.dma_start(wxb_sb[:],
                                          src.rearrange("t p c -> p t c"))
                        hstage = hsp.tile([P, G, CB], bf16, name="hstage")
                        pstage = psp.tile([P, G, CB], f32, name="pstage")
                    puA, puB = pua[t % 2], pub[t % 2]
                    ws = wxb_sb[:, t % G, :]
                    # preload Wx+b into PSUM (MMs accumulate onto it)
                    nc.vector.tensor_copy(puA[:], ws[:, 0:H])
                    nc.vector.tensor_copy(puB[:], ws[:, H:CB])
                    newh = hstage[:, t % G, :]
                    # two d_in blocks; within a block d_out-major so each half
                    # of pu completes early in block 2
                    for blk in range(2):
                        for do in range(DC):
                            pu = puA if do < 4 else puB
                            out = pu[:, (do % 4) * B:(do % 4 + 1) * B]
                            for di in range(blk * 4, blk * 4 + 4):
                                nc.tensor.matmul(
                                    out,
                                    wsT_sb[:, di, do * P:(do + 1) * P],
                                    hbf[:, di * B:(di + 1) * B],
                                    start=False,
                                    stop=(blk == 1 and di == blk * 4 + 3),
                                    skip_group_check=True)
                            if blk == 1 and do == 3:
                                nc.scalar.activation(newh[:, 0:H], puA[:],
                                                     AF.Tanh)
                        if blk == 1:
                            nc.scalar.activation(newh[:, H:CB], puB[:],
                                                 AF.Tanh)
                    nc.vector.tensor_scalar_mul(pstage[:, t % G, :], newh[:],
                                                c_sb[:])
                    hbf = newh
                    if (t + 1) % G == 0:
                        g8 = t // G
                        nc.sync.dma_start(hsh[:, t - G + 1:t + 1, :],
                                          hstage[:])
                        nc.sync.dma_start(
                            csin[ck][(t - G + 1 - ck_off) // G], pstage[:])
                    if t - ck_off + 1 == CHUNKS[ck]:
                        nc.gpsimd.collective_compute(
                            "ReduceScatter", mybir.AluOpType.add,
                            replica_groups=rg,
                            ins=[csin[ck][:]], outs=[csout[ck][:]])
                        ck_off += CHUNKS[ck]
                        ck += 1

            # ============== out = silu(z) * hsum (post-loop) ==============
            with (
                tc.tile_pool(name="cz", bufs=3) as czp,
                tc.tile_pool(name="ch", bufs=3) as chp,
                tc.tile_pool(name="co", bufs=3) as cop,
            ):
                r0 = 0
                for k, ln in enumerate(CHUNKS):
                    ngrp = ln // G // NCORES
                    for g in range(ngrp):
                        hz = chp.tile([P, G, CB], f32, name="hz")
                        nc.sync.dma_start(hz[:], csout[k][g])
                        zz = czp.tile([P, G, CB], f32, name="zz")
                        nc.sync.dma_start(zz[:], zt[r0 + g])
                        sz = cop.tile([P, G, CB], f32, name="sz")
                        nc.scalar.activation(sz[:], zz[:], AF.Silu)
                        oo = cop.tile([P, G, CB], f32, name="oo")
                        nc.vector.tensor_mul(oo[:], sz[:], hz[:])
                        nc.sync.dma_start(osh[r0 + g], oo[:])
                    r0 += ngrp

    nc.compile()
    return nc


_NC_CACHE = {}
_LAST_IN_MAPS = None


def _get_nc():
    if "nc" not in _NC_CACHE:
        _NC_CACHE["nc"] = _build()
    return _NC_CACHE["nc"]


def _pack_bdp(a):
    """[..., B, D] -> [..., P, CB] tile layout: tile[p, dc*B+b] = a[b, dc*P+p]."""
    lead = a.shape[:-2]
    a = a.reshape(lead + (B, DC, P))
    n = len(lead)
    order = tuple(range(n)) + (n + 2, n + 1, n)
    return np.ascontiguousarray(a.transpose(order)).reshape(lead + (P, CB))


def _unpack_bdp(a):
    """[..., P, CB] -> [..., B, D]."""
    lead = a.shape[:-2]
    a = a.reshape(lead + (P, DC, B))
    n = len(lead)
    order = tuple(range(n)) + (n + 2, n + 1, n)
    return np.ascontiguousarray(a.transpose(order)).reshape(lead + (B, D))


def kernel(x, z, h0, W_x, U, V, b, C):
    from concourse.bass_utils import run_bass_kernel_spmd

    x = np.asarray(x, np.float32)
    z = np.asarray(z, np.float32)
    h0 = np.asarray(h0, np.float32)
    W_x = np.asarray(W_x, np.float32)
    U = np.asarray(U, np.float32)
    V = np.asarray(V, np.float32)
    b = np.asarray(b, np.float32)
    C = np.asarray(C, np.float32)

    nc = _get_nc()

    xT_np = np.ascontiguousarray(x.reshape(T * B, D).T)
    wxT_np = np.ascontiguousarray(W_x.T)
    bt_np = np.ascontiguousarray(b.reshape(DC, P).T)

    in_maps = []
    for s in range(NCORES):
        tix = _t_indices(s)
        zp = _pack_bdp(z[tix])                      # [TSH, P, CB]
        zp = np.ascontiguousarray(
            zp.reshape(TSH // G, G, P, CB).transpose(0, 2, 1, 3))
        in_maps.append({
            "xT0": np.ascontiguousarray(xT_np[:, :TSHARD * B]),
            "xTm": np.ascontiguousarray(
                xT_np[:, s * TSHARD * B:(s + 1) * TSHARD * B]),
            "wxT": wxT_np,
            "vnat": V[s].astype(BF16),
            "uT": np.ascontiguousarray(U[s].T).astype(BF16),
            "bt": bt_np,
            "cst": np.full((P, 1), C[s], np.float32),
            "h0b": _pack_bdp(h0[:, s, :]).astype(BF16),
            "zt": zp,
        })

    global _LAST_IN_MAPS
    _LAST_IN_MAPS = in_maps
    res = run_bass_kernel_spmd(nc, in_maps, core_ids=list(range(NCORES)))

    h = np.empty((T + 1, S, B, D), np.float32)
    out = np.empty((T, B, D), np.float32)
    for s in range(NCORES):
        h[0, s] = h0[:, s, :]
        hp = res.results[s]["hsh"].astype(np.float32)   # [P, T, CB]
        h[1:, s] = _unpack_bdp(np.ascontiguousarray(hp.transpose(1, 0, 2)))
        op = res.results[s]["osh"]                      # [TSH//G, P, G, CB]
        op = np.ascontiguousarray(op.transpose(0, 2, 1, 3)).reshape(
            TSH, P, CB)
        out[_t_indices(s)] = _unpack_bdp(op)
    return out, h
